# revision 1
# baseline (speedup 1.0000x reference)
"""Multi-head attention (B=2, N=2048, C=768, H=12) on 8 trn2 cores.

Sharding: core i handles batch b = i//4 and head-group g = i%4 (3 heads).
All device data is fp16 (tolerance 2e-2 allows it); matmul accumulation
stays fp32 in PSUM.

Per-core pipeline:
  1. QKV^T projection from host-pre-transposed xT [C, N]:
       q01/k01  [128, N]: heads 0,1 d-major (h0 at partitions 0:64, h1 at
                64:128) -> natural row-tile pairing for the score matmuls.
       q2d/k2d  [128, N]: head 2 duplicated in both partition halves so its
                score matmuls can be row-tile paired across adjacent k-chunks.
       v        [N, 65] per (k-chunk, head): cols 0:64 = v, col 64 = ones
                (softmax denominator trick).
  2. Scores transposed: S^T[k, q] = k_h^T-chunk.T @ q_h. Heads 0/1 (and for
     head 2, adjacent k-chunks) issue as K=64 matmuls at tile_position
     (0,0)/(64,0) -> they stream concurrently in the PE array.
  3. exp via ScalarE (the kernel's throughput floor: ~96 activations of
     [128,1024]); output fp16 to SBUF.
  4. attn@V with lhsT = [v | 1]: psum rows 0:64 = unnormalized attn_out^T,
     row 64 = denominators.  Normalize: gpsimd partition-broadcast of the
     denominator row, DVE reciprocal_approx_fast, DVE multiply -> fp16.
  5. Output projection LOCALLY (w_proj row-shard, 3 K=64 chunks + bias),
     then a 4-core ReduceScatter(add) per q-window delivers each core its
     final [192, N] output shard.  Window 0's RS overlaps window 1 compute.

Scheduling: the whole kernel is one software pipeline in 4 phases
(w0-h2, w0-h01, w1-h2, w1-h01).  QKV projection groups, v-projection
chunks, and window-0 proj chunks are interleaved into the attention
phases as PE gap fillers (the PE otherwise idles under the ScalarE exp
pace and the HAM clock gate then halves its clock).  attn@V accums
trail the score/exp stream by two k-chunks so the in-order PE queue
never head-of-line blocks on an accumulator psum slot that waits for
the previous phase's normalize.  A tiny warmup collective at kernel
start absorbs the ~11us first-collective trigger overhead.
"""

import numpy as np

B, N, C, H, HD = 2, 2048, 768, 12, 64
G = 4              # tensor-parallel head groups
HL = H // G        # 3 heads per core
CHL = HL * HD      # 192 local channels
SCALE = HD ** -0.5
NCORES = 8
CT = C // 128      # 6 contraction chunks
FW = 512           # matmul free width (psum bank)
QW = 1024          # q window width
NWIN = N // QW     # 2 windows
KT = N // 128      # 16 k chunks
VW = HD + 1        # v tile cols: 64 v + 1 ones

_CACHE = {}


def _build_nc():
    import concourse.bass as bass
    import concourse.bacc as bacc
    import concourse.tile as tile
    import concourse.mybir as mybir

    F32 = mybir.dt.float32
    F16 = mybir.dt.float16
    AF = mybir.ActivationFunctionType
    RG = [[0, 1, 2, 3], [4, 5, 6, 7]]

    nc = bacc.Bacc(num_devices=NCORES)
    xT_d = nc.declare_dram_parameter("xT", [C, N], F16, isOutput=False)
    wqa_d = nc.declare_dram_parameter("wqa", [C, 256], F16, isOutput=False)
    wka_d = nc.declare_dram_parameter("wka", [C, 256], F16, isOutput=False)
    wv_d = nc.declare_dram_parameter("wv", [C, CHL], F16, isOutput=False)
    wp_d = nc.declare_dram_parameter("wp", [CHL, C], F16, isOutput=False)
    bp_d = nc.declare_dram_parameter("bp", [1, C], F16, isOutput=False)
    out_d = nc.declare_dram_parameter("out", [CHL, N], F16, isOutput=True)

    with tile.TileContext(nc) as tc:
        with tc.tile_pool(name="dram", bufs=1, space="DRAM") as dram:
            rs_ins = [dram.tile([C, QW], F16, name=f"rs_in{w}")
                      for w in range(NWIN)]
            rs_outs = [dram.tile([CHL, QW], F16, name=f"rs_out{w}")
                       for w in range(NWIN)]
            warm_in = dram.tile([8, 128], F16, name="warm_in")
            warm_out = dram.tile([2, 128], F16, name="warm_out")

            with tc.tile_pool(name="sb", bufs=1) as P, \
                    tc.tile_pool(name="ps", bufs=1, space="PSUM") as PS:
                # ---- input DMAs (weights first: small, unblock matmuls) ----
                wka_sb = P.tile([128, CT, 256], F16)
                wqa_sb = P.tile([128, CT, 256], F16)
                for ct in range(CT):
                    nc.sync.dma_start(out=wka_sb[:, ct, :],
                                      in_=wka_d[ct * 128:(ct + 1) * 128, :])
                xT_sb = P.tile([128, CT, N], F16)
                for ct in range(CT):
                    nc.sync.dma_start(out=xT_sb[:, ct, :],
                                      in_=xT_d[ct * 128:(ct + 1) * 128, :])
                for ct in range(CT):
                    nc.sync.dma_start(out=wqa_sb[:, ct, :],
                                      in_=wqa_d[ct * 128:(ct + 1) * 128, :])
                wv_sb = P.tile([128, CT, CHL], F16)
                for ct in range(CT):
                    nc.sync.dma_start(out=wv_sb[:, ct, :],
                                      in_=wv_d[ct * 128:(ct + 1) * 128, :])
                # w_proj row-shard, one [64, C] tile per local head
                wp_sb = P.tile([64, HL, C], F16)
                for h in range(HL):
                    nc.sync.dma_start(out=wp_sb[:, h, :],
                                      in_=wp_d[h * 64:(h + 1) * 64, :])
                bp_sb = P.tile([1, C], F16)
                nc.sync.dma_start(out=bp_sb[:], in_=bp_d[:, :])
                ones_q = P.tile([1, FW], F16)
                nc.vector.memset(ones_q[:], 1.0)
                warm_sb = P.tile([8, 128], F16)

                # ---- persistent QKV results ----
                k01_sb = P.tile([128, N], F16)
                q01_sb = P.tile([128, N], F16)
                k2d_sb = P.tile([128, N], F16)
                q2d_sb = P.tile([128, N], F16)
                # [n, kt, h, VW]: cols 0:64 = v, col 64 = ones
                v_sb = P.tile([128, KT, HL, VW], F16)
                nc.vector.memset(v_sb[:, :, :, HD:VW], 1.0)

                # ---- QKV projection emitters (interleaved into attention
                # as PE gap-fillers: keeps TensorE dense so HAM stays at
                # full clock) ----
                # psum ring "sc": [128,1024] slots (2 banks) x2 -> qk/v
                # projection psum, score tiles, and proj psum.
                # psum ring "ac": [128,1024] slots x2 -> attention
                # accumulators only.
                def qk_group(dst, wsb, mlo, f):
                    qk_ps = PS.tile([128, FW], F32, tag="sc", bufs=2,
                                    padded_shape=[128, QW], name="qk_ps")
                    for ct in range(CT):
                        nc.tensor.matmul(
                            qk_ps[:],
                            lhsT=wsb[:, ct, mlo:mlo + 128],
                            rhs=xT_sb[:, ct, f * FW:(f + 1) * FW],
                            start=(ct == 0), stop=(ct == CT - 1),
                        )
                    nc.vector.tensor_copy(
                        dst[:, f * FW:(f + 1) * FW], qk_ps[:])

                def v_group(nt):
                    v_ps = PS.tile([128, CHL], F32, tag="sc", bufs=2,
                                   padded_shape=[128, QW], name="v_ps")
                    for ct in range(CT):
                        nc.tensor.matmul(
                            v_ps[:],
                            lhsT=xT_sb[:, ct, nt * 128:(nt + 1) * 128],
                            rhs=wv_sb[:, ct, :],
                            start=(ct == 0), stop=(ct == CT - 1),
                        )
                    nc.vector.tensor_copy(
                        v_sb[:, nt, :, 0:HD],
                        v_ps[:].rearrange("p (h d) -> p h d", h=HL))

                def dummy_burst(S, n):
                    """Throwaway matmuls into an already-consumed score tile:
                    keeps the PE array busy across pipeline gaps so the HAM
                    clock gate stays at full rate."""
                    for _ in range(n):
                        nc.tensor.matmul(
                            S[:, 0:FW],
                            lhsT=k01_sb[0:64, 0:128],
                            rhs=k01_sb[0:64, 0:FW],
                        )

                # tiny warmup collective: absorbs the ~11us first-collective
                # start overhead long before the real ReduceScatters.
                nc.vector.memset(warm_sb[:], 0.0)
                nc.sync.dma_start(out=warm_in[:], in_=warm_sb[:])
                nc.gpsimd.collective_compute(
                    "ReduceScatter",
                    mybir.AluOpType.add,
                    replica_groups=RG,
                    ins=[warm_in.opt()],
                    outs=[warm_out.opt()],
                )

                # prologue: only what the first score pair needs (k2d f0
                # covers k-chunks 0-3; q2d f0/f1 cover window 0); k2d f1-3
                # ride as early phase-A fillers, 2 iterations ahead of use
                qk_group(k2d_sb, wka_sb, 128, 0)
                qk_group(q2d_sb, wqa_sb, 128, 0)
                qk_group(q2d_sb, wqa_sb, 128, 1)

                # ---- attention + local proj + per-window ReduceScatter ----
                with tc.tile_pool(name="att_sb", bufs=1) as AS:
                    def scores_pair(w, lhs_tile, rhs_tile, kc0, kc1, Sa, Sb):
                        """Two K=64 score matmuls row-tiled (0,0)/(64,0)."""
                        q0 = w * QW
                        for j in range(QW // FW):
                            js = slice(q0 + j * FW, q0 + (j + 1) * FW)
                            ps_js = slice(j * FW, (j + 1) * FW)
                            nc.tensor.matmul(
                                Sa[:, ps_js],
                                lhsT=lhs_tile[0:64, kc0 * 128:(kc0 + 1) * 128],
                                rhs=rhs_tile[0:64, js],
                            )
                            nc.tensor.matmul(
                                Sb[:, ps_js],
                                lhsT=lhs_tile[64:128, kc1 * 128:(kc1 + 1) * 128],
                                rhs=rhs_tile[64:128, js],
                            )

                    def av_accum(A, E, kc, h, first, last):
                        for j in range(QW // FW):
                            ps_js = slice(j * FW, (j + 1) * FW)
                            nc.tensor.matmul(
                                A[:, ps_js],
                                lhsT=v_sb[:, kc, h, :],
                                rhs=E[:, ps_js],
                                start=first, stop=last,
                            )

                    def normalize(A, at):
                        """at[0:64] = A[0:64] / A[64] (denominator row)."""
                        for j in range(QW // FW):
                            js = slice(j * FW, (j + 1) * FW)
                            den = AS.tile([1, FW], F32, tag="den", bufs=4)
                            bcs = AS.tile([64, FW], F32, tag="bcs", bufs=4)
                            rcp = AS.tile([64, FW], F32, tag="rcp", bufs=4)
                            nc.vector.tensor_copy(den[:], A[64:65, js])
                            nc.gpsimd.partition_broadcast(bcs[:], den[:])
                            nc.vector.reciprocal_approx_fast(rcp[:], bcs[:])
                            nc.vector.tensor_mul(at[:, js], A[0:64, js],
                                                 rcp[:])

                    def proj_chunk(w, ats, m):
                        """out^T[m-chunk, w] partial = sum_h wp_h.T@at_h + b."""
                        pr = PS.tile([128, QW], F32, tag="sc", bufs=2)
                        ms = slice(m * 128, (m + 1) * 128)
                        for j in range(QW // FW):
                            ps_js = slice(j * FW, (j + 1) * FW)
                            for h in range(HL):
                                nc.tensor.matmul(
                                    pr[:, ps_js],
                                    lhsT=wp_sb[:, h, ms],
                                    rhs=ats[h][:, ps_js],
                                    start=(h == 0), stop=False,
                                )
                            nc.tensor.matmul(
                                pr[:, ps_js],
                                lhsT=bp_sb[:, ms],
                                rhs=ones_q[:],
                                start=False, stop=True,
                            )
                        po = AS.tile([128, QW], F16, tag="po", bufs=3)
                        nc.vector.tensor_copy(po[:], pr[:])
                        nc.sync.dma_start(out=rs_ins[w][ms, :], in_=po[:])

                    def reduce_scatter(ww):
                        nc.gpsimd.collective_compute(
                            "ReduceScatter",
                            mybir.AluOpType.add,
                            replica_groups=RG,
                            ins=[rs_ins[ww].opt()],
                            outs=[rs_outs[ww].opt()],
                        )
                        nc.sync.dma_start(
                            out=out_d[:, ww * QW:(ww + 1) * QW],
                            in_=rs_outs[ww][:, :],
                        )

                    def attn_h2(w, interleave):
                        """Head 2, adjacent-k-chunk row-tile-paired.
                        av_accums trail the scores/exp stream by one kcp so
                        the in-order PE queue never head-of-line blocks on
                        the accumulator psum slot (which waits for the
                        previous phase's normalize) at a phase seam."""
                        A2 = PS.tile([VW, QW], F32, tag="ac", bufs=2,
                                     padded_shape=[128, QW], name=f"A2_{w}")
                        pend = []
                        for kcp in range(KT // 2):
                            kc0, kc1 = 2 * kcp, 2 * kcp + 1
                            Se = PS.tile([128, QW], F32, tag="sc", bufs=2)
                            So = PS.tile([128, QW], F32, tag="sc", bufs=2)
                            scores_pair(w, k2d_sb, q2d_sb, kc0, kc1, Se, So)
                            Ee = AS.tile([128, QW], F16, tag="E", bufs=10)
                            Eo = AS.tile([128, QW], F16, tag="E", bufs=10)
                            nc.scalar.activation(Ee[:], Se[:], AF.Exp,
                                                 scale=SCALE)
                            nc.scalar.activation(Eo[:], So[:], AF.Exp,
                                                 scale=SCALE)
                            pend += [(Ee, kc0), (Eo, kc1)]
                            interleave(kcp)
                            while len(pend) > 2:
                                E, kc = pend.pop(0)
                                av_accum(A2, E, kc, 2, kc == 0, kc == KT - 1)
                        for E, kc in pend:
                            av_accum(A2, E, kc, 2, kc == 0, kc == KT - 1)
                        return A2

                    def attn_h01(w, interleave):
                        """Heads 0/1, head-row-tile-paired; accums trail by
                        two k-chunks (see attn_h2)."""
                        A0 = PS.tile([VW, QW], F32, tag="ac", bufs=2,
                                     padded_shape=[128, QW], name=f"A0_{w}")
                        A1 = PS.tile([VW, QW], F32, tag="ac", bufs=2,
                                     padded_shape=[128, QW], name=f"A1_{w}")
                        pend = []
                        for kc in range(KT):
                            S0 = PS.tile([128, QW], F32, tag="sc", bufs=2)
                            S1 = PS.tile([128, QW], F32, tag="sc", bufs=2)
                            scores_pair(w, k01_sb, q01_sb, kc, kc, S0, S1)
                            E0 = AS.tile([128, QW], F16, tag="E", bufs=10)
                            E1 = AS.tile([128, QW], F16, tag="E", bufs=10)
                            nc.scalar.activation(E0[:], S0[:], AF.Exp,
                                                 scale=SCALE)
                            nc.scalar.activation(E1[:], S1[:], AF.Exp,
                                                 scale=SCALE)
                            pend.append((E0, E1, kc))
                            interleave(kc)
                            while len(pend) > 2:
                                E0p, E1p, kcp_ = pend.pop(0)
                                av_accum(A0, E0p, kcp_, 0, kcp_ == 0,
                                         kcp_ == KT - 1)
                                av_accum(A1, E1p, kcp_, 1, kcp_ == 0,
                                         kcp_ == KT - 1)
                        for E0p, E1p, kcp_ in pend:
                            av_accum(A0, E0p, kcp_, 0, kcp_ == 0,
                                     kcp_ == KT - 1)
                            av_accum(A1, E1p, kcp_, 1, kcp_ == 0,
                                     kcp_ == KT - 1)
                        return A0, A1

                    ats0 = [AS.tile([64, QW], F16, tag=f"at{h}", bufs=2,
                                    name=f"at{h}_w0")
                            for h in range(HL)]
                    ats1 = [AS.tile([64, QW], F16, tag=f"at{h}", bufs=2,
                                    name=f"at{h}_w1")
                            for h in range(HL)]

                    # phase A: w0 head 2; all v chunks + phase-B q/k deps as
                    # fillers.  Accums for kc pair j flush at iteration j+1,
                    # after interleave(j+1), so v[2j]/v[2j+1] (supplied at
                    # iteration j) always precede their consumption.
                    def fillA(kcp):
                        v_group(2 * kcp)
                        v_group(2 * kcp + 1)
                        if kcp < 3:
                            # k2d f(kcp+1): consumed by score pairs at
                            # iteration 2*(kcp+1), two iterations later
                            qk_group(k2d_sb, wka_sb, 128, kcp + 1)
                        elif kcp == 3:
                            qk_group(k01_sb, wka_sb, 0, 0)
                        elif kcp == 4:
                            qk_group(q01_sb, wqa_sb, 0, 0)
                        elif kcp == 5:
                            qk_group(q01_sb, wqa_sb, 0, 1)
                    A2 = attn_h2(0, fillA)
                    # phase B: w0 heads 0/1; rest of the q/k projections
                    normalize(A2, ats0[2])
                    fillB = [
                        lambda: qk_group(k01_sb, wka_sb, 0, 1),
                        lambda: qk_group(k01_sb, wka_sb, 0, 2),
                        lambda: qk_group(k01_sb, wka_sb, 0, 3),
                        lambda: qk_group(q01_sb, wqa_sb, 0, 2),
                        lambda: qk_group(q01_sb, wqa_sb, 0, 3),
                        lambda: qk_group(q2d_sb, wqa_sb, 128, 2),
                        lambda: qk_group(q2d_sb, wqa_sb, 128, 3),
                    ]
                    A0, A1 = attn_h01(
                        0, lambda kc: fillB[kc]() if kc < len(fillB) else None)
                    # phase C: w1 head 2; 3 window-0 proj chunks as fillers
                    normalize(A0, ats0[0])
                    normalize(A1, ats0[1])
                    A2 = attn_h2(
                        1, lambda kcp: proj_chunk(0, ats0, (kcp - 2) // 2)
                        if kcp in (2, 4, 6) else None)
                    # phase D: w1 heads 0/1; 3 more w0 proj chunks, then RS(0)
                    normalize(A2, ats1[2])

                    def fillD(kc):
                        if kc in (0, 2, 4):
                            proj_chunk(0, ats0, 3 + kc // 2)
                        elif kc == 5:
                            reduce_scatter(0)
                    A0, A1 = attn_h01(1, fillD)

                    # ---- tail: pipelined normalize + window-1 proj ----
                    # A contiguous dummy burst bridges the PE across the
                    # normalize chains (ACT is done, so no power contention)
                    # and re-arms the HAM un-throttle for the projection.
                    dumT = PS.tile([128, QW], F32, tag="sc", bufs=2)
                    dummy_burst(dumT, 18)
                    # First two m-chunks are emitted head-by-head so their
                    # head-2/head-0 matmuls overlap the remaining normalizes;
                    # psum->sbuf copies ride the now-idle scalar engine.
                    def pr_head_mms(pr, m, h, start, stop, bias=False):
                        ms = slice(m * 128, (m + 1) * 128)
                        for j in range(QW // FW):
                            ps_js = slice(j * FW, (j + 1) * FW)
                            nc.tensor.matmul(
                                pr[:, ps_js],
                                lhsT=wp_sb[:, h, ms],
                                rhs=ats1[h][:, ps_js],
                                start=start, stop=False,
                            )
                            if bias:
                                nc.tensor.matmul(
                                    pr[:, ps_js],
                                    lhsT=bp_sb[:, ms],
                                    rhs=ones_q[:],
                                    start=False, stop=stop,
                                )

                    def pr_finish(pr, m):
                        po = AS.tile([128, QW], F16, tag="po", bufs=3)
                        nc.scalar.activation(po[:], pr[:], AF.Copy)
                        nc.sync.dma_start(
                            out=rs_ins[1][m * 128:(m + 1) * 128, :], in_=po[:])

                    pr0 = PS.tile([128, QW], F32, tag="sc", bufs=2)
                    pr1 = PS.tile([128, QW], F32, tag="sc", bufs=2)
                    pr_head_mms(pr0, 0, 2, True, False)
                    pr_head_mms(pr1, 1, 2, True, False)
                    normalize(A0, ats1[0])
                    pr_head_mms(pr0, 0, 0, False, False)
                    pr_head_mms(pr1, 1, 0, False, False)
                    normalize(A1, ats1[1])
                    pr_head_mms(pr0, 0, 1, False, True, bias=True)
                    pr_finish(pr0, 0)
                    pr_head_mms(pr1, 1, 1, False, True, bias=True)
                    pr_finish(pr1, 1)
                    for m in range(2, C // 128):
                        pr = PS.tile([128, QW], F32, tag="sc", bufs=2)
                        pr_head_mms(pr, m, 2, True, False)
                        pr_head_mms(pr, m, 0, False, False)
                        pr_head_mms(pr, m, 1, False, True, bias=True)
                        pr_finish(pr, m)
                    reduce_scatter(1)
    nc.finalize()
    return nc


def get_nc():
    if "nc" not in _CACHE:
        _CACHE["nc"] = _build_nc()
    return _CACHE["nc"]


def make_in_maps(x, w_qkv, w_proj, b_proj):
    x = np.asarray(x, dtype=np.float32)
    w_qkv = np.asarray(w_qkv, dtype=np.float32)
    w_proj = np.asarray(w_proj, dtype=np.float32)
    b_proj = np.asarray(b_proj, dtype=np.float32)
    in_maps = []
    for core in range(NCORES):
        b, g = divmod(core, G)
        cs = slice(g * CHL, (g + 1) * CHL)
        wq = w_qkv[:, 0 * C:1 * C][:, cs]
        wk = w_qkv[:, 1 * C:2 * C][:, cs]
        wv = w_qkv[:, 2 * C:3 * C][:, cs]
        # [heads01 | head2 | head2-dup]
        wqa = np.concatenate([wq[:, 0:128], wq[:, 128:192], wq[:, 128:192]],
                             axis=1)
        wka = np.concatenate([wk[:, 0:128], wk[:, 128:192], wk[:, 128:192]],
                             axis=1)
        bp = b_proj if g == 0 else np.zeros_like(b_proj)
        im = {
            "xT": np.ascontiguousarray(x[b].T, dtype=np.float16),
            "wqa": np.ascontiguousarray(wqa, dtype=np.float16),
            "wka": np.ascontiguousarray(wka, dtype=np.float16),
            "wv": np.ascontiguousarray(wv, dtype=np.float16),
            "wp": np.ascontiguousarray(w_proj[cs, :], dtype=np.float16),
            "bp": np.ascontiguousarray(bp.reshape(1, C), dtype=np.float16),
        }
        in_maps.append(im)
    return in_maps


def unshard(results):
    out = np.empty((B, N, C), dtype=np.float32)
    for b in range(B):
        outT = np.concatenate(
            [np.asarray(results[b * G + g]["out"], dtype=np.float32)
             for g in range(G)], axis=0)
        out[b] = outT.T
    return out


def kernel(x, w_qkv, w_proj, b_proj):
    from concourse.bass_utils import run_bass_kernel_spmd

    nc = get_nc()
    in_maps = make_in_maps(x, w_qkv, w_proj, b_proj)
    res = run_bass_kernel_spmd(nc, in_maps, list(range(NCORES)))
    return unshard(res.results)



# revision 2
# speedup vs baseline: 1.2020x; 1.2020x over previous
"""Multi-head attention (B=2, N=2048, C=768, H=12) on 8 trn2 cores.

Sharding: core i handles batch b = i//4 and head-group g = i%4 (3 heads).
All device data is fp16 (tolerance 2e-2 allows it); matmul accumulation
stays fp32 in PSUM.

Per-core pipeline:
  1. QKV^T projection from host-pre-transposed xT [C, N]:
       q01/k01  [128, N]: heads 0,1 d-major (h0 at partitions 0:64, h1 at
                64:128) -> natural row-tile pairing for the score matmuls.
       q2d/k2d  [128, N]: head 2 duplicated in both partition halves so its
                score matmuls can be row-tile paired across adjacent k-chunks.
       v        [N, 65] per (k-chunk, head): cols 0:64 = v, col 64 = ones
                (softmax denominator trick).
  2. Scores transposed: S^T[k, q] = k_h^T-chunk.T @ q_h. Heads 0/1 (and for
     head 2, adjacent k-chunks) issue as K=64 matmuls at tile_position
     (0,0)/(64,0) -> they stream concurrently in the PE array.
  3. exp via ScalarE (the kernel's throughput floor: ~96 activations of
     [128,1024]); output fp16 to SBUF.
  4. attn@V with lhsT = [v | 1]: psum rows 0:64 = unnormalized attn_out^T,
     row 64 = denominators.  Normalize: gpsimd partition-broadcast of the
     denominator row, DVE reciprocal_approx_fast, DVE multiply -> fp16.
  5. Output exchange via per-window AllGather of the NORMALIZED attention
     output at [192, 1024] (4x less data than ReduceScattering projection
     partials).  The gathered at_full [768, 1024] has c_in rows in natural
     order (group g rows at g*192 = heads 3g..3g+2), so each core then
     computes its c_out slice of the projection locally at full K=128
     efficiency: out^T[cs, q] = w_proj[:, cs].T @ at_full + b[cs], with the
     bias folded into the psum->sbuf copy (DVE tensor_scalar add).
     AG(w0) triggers at phase C start and hides under phases C/D; its
     projection chunks run as phase-D PE fillers.  Only AG(w1) + a ~7us
     local projection tail is exposed.

Scheduling: attention is one software pipeline in 4 phases (w0-h2,
w0-h01, w1-h2, w1-h01).  QKV projection groups and v-projection chunks
interleave into phases A/B as PE gap fillers (the PE otherwise idles
under the ScalarE exp pace and the HAM clock gate then halves its
clock).  attn@V accums trail the score/exp stream by two k-chunks so
the in-order PE queue never head-of-line blocks on an accumulator psum
slot that waits for the previous phase's normalize.  A tiny warmup
collective at kernel start absorbs the ~60us first-collective CC-core
boot latency.  Input DMAs are f-major so the first projection groups
unblock after ~1MB instead of the full 3.1MB xT load.
"""

import numpy as np

B, N, C, H, HD = 2, 2048, 768, 12, 64
G = 4              # tensor-parallel head groups
HL = H // G        # 3 heads per core
CHL = HL * HD      # 192 local channels
SCALE = HD ** -0.5
NCORES = 8
CT = C // 128      # 6 contraction chunks
FW = 512           # matmul free width (psum bank)
QW = 1024          # q window width
NWIN = N // QW     # 2 windows
KT = N // 128      # 16 k chunks
VW = HD + 1        # v tile cols: 64 v + 1 ones

_CACHE = {}


def _build_nc():
    import concourse.bass as bass
    import concourse.bacc as bacc
    import concourse.tile as tile
    import concourse.mybir as mybir

    F32 = mybir.dt.float32
    F16 = mybir.dt.float16
    AF = mybir.ActivationFunctionType
    RG = [[0, 1, 2, 3], [4, 5, 6, 7]]

    nc = bacc.Bacc(num_devices=NCORES)
    xT_d = nc.declare_dram_parameter("xT", [C, N], F16, isOutput=False)
    wqa_d = nc.declare_dram_parameter("wqa", [C, 256], F16, isOutput=False)
    wka_d = nc.declare_dram_parameter("wka", [C, 256], F16, isOutput=False)
    wv_d = nc.declare_dram_parameter("wv", [C, CHL], F16, isOutput=False)
    wp_d = nc.declare_dram_parameter("wp", [C, CHL], F16, isOutput=False)
    bp_d = nc.declare_dram_parameter("bp", [128, 2], F32, isOutput=False)
    out_d = nc.declare_dram_parameter("out", [CHL, N], F16, isOutput=True)

    with tile.TileContext(nc) as tc:
        with tc.tile_pool(name="dram", bufs=1, space="DRAM") as dram:
            ag_ins = [dram.tile([CHL, QW], F16, name=f"ag_in{w}")
                      for w in range(NWIN)]
            ag_outs = [dram.tile([C, QW], F16, name=f"ag_out{w}")
                       for w in range(NWIN)]
            warm_in = dram.tile([8, 128], F16, name="warm_in")
            warm_out = dram.tile([32, 128], F16, name="warm_out")

            with tc.tile_pool(name="sb", bufs=1) as P, \
                    tc.tile_pool(name="ps", bufs=1, space="PSUM") as PS:
                # ---- input DMAs, f-major on xT: the first projection
                # groups only need xT f-chunk 0/1, so they unblock after
                # ~1MB instead of the full 3.1MB load ----
                wka_sb = P.tile([128, CT, 256], F16)
                wqa_sb = P.tile([128, CT, 256], F16)
                xT_sb = P.tile([128, CT, N], F16)
                for ct in range(CT):
                    nc.sync.dma_start(out=wka_sb[:, ct, :],
                                      in_=wka_d[ct * 128:(ct + 1) * 128, :])
                for ct in range(CT):
                    nc.sync.dma_start(
                        out=xT_sb[:, ct, 0:FW],
                        in_=xT_d[ct * 128:(ct + 1) * 128, 0:FW])
                for ct in range(CT):
                    nc.sync.dma_start(out=wqa_sb[:, ct, :],
                                      in_=wqa_d[ct * 128:(ct + 1) * 128, :])
                for f in range(1, N // FW):
                    fs = slice(f * FW, (f + 1) * FW)
                    for ct in range(CT):
                        nc.sync.dma_start(
                            out=xT_sb[:, ct, fs],
                            in_=xT_d[ct * 128:(ct + 1) * 128, fs])
                wv_sb = P.tile([128, CT, CHL], F16)
                for ct in range(CT):
                    nc.sync.dma_start(out=wv_sb[:, ct, :],
                                      in_=wv_d[ct * 128:(ct + 1) * 128, :])
                # w_proj column-slice [C, 192], row chunks ct
                wp_sb = P.tile([128, CT, CHL], F16)
                for ct in range(CT):
                    nc.sync.dma_start(out=wp_sb[:, ct, :],
                                      in_=wp_d[ct * 128:(ct + 1) * 128, :])
                bp_sb = P.tile([128, 2], F32)
                nc.sync.dma_start(out=bp_sb[:], in_=bp_d[:, :])
                warm_sb = P.tile([8, 128], F16)

                # ---- persistent QKV results ----
                k01_sb = P.tile([128, N], F16)
                q01_sb = P.tile([128, N], F16)
                k2d_sb = P.tile([128, N], F16)
                q2d_sb = P.tile([128, N], F16)
                # [n, kt, h, VW]: cols 0:64 = v, col 64 = ones
                v_sb = P.tile([128, KT, HL, VW], F16)
                nc.vector.memset(v_sb[:, :, :, HD:VW], 1.0)
                # gathered attention output, [c_in chunk, kc, q]
                atf_sb = [P.tile([128, CT, QW], F16, name=f"atf{w}")
                          for w in range(NWIN)]

                # ---- QKV projection emitters (interleaved into attention
                # as PE gap-fillers: keeps TensorE dense so HAM stays at
                # full clock) ----
                # psum ring "sc": [128,1024] slots (2 banks) x2 -> qk/v
                # projection psum, score tiles, and proj psum.
                # psum ring "ac": [128,1024] slots x2 -> attention
                # accumulators only.
                def qk_group(dst, wsb, mlo, f):
                    qk_ps = PS.tile([128, FW], F32, tag="sc", bufs=2,
                                    padded_shape=[128, QW], name="qk_ps")
                    for ct in range(CT):
                        nc.tensor.matmul(
                            qk_ps[:],
                            lhsT=wsb[:, ct, mlo:mlo + 128],
                            rhs=xT_sb[:, ct, f * FW:(f + 1) * FW],
                            start=(ct == 0), stop=(ct == CT - 1),
                        )
                    nc.vector.tensor_copy(
                        dst[:, f * FW:(f + 1) * FW], qk_ps[:])

                def v_group(nt):
                    v_ps = PS.tile([128, CHL], F32, tag="sc", bufs=2,
                                   padded_shape=[128, QW], name="v_ps")
                    for ct in range(CT):
                        nc.tensor.matmul(
                            v_ps[:],
                            lhsT=xT_sb[:, ct, nt * 128:(nt + 1) * 128],
                            rhs=wv_sb[:, ct, :],
                            start=(ct == 0), stop=(ct == CT - 1),
                        )
                    nc.vector.tensor_copy(
                        v_sb[:, nt, :, 0:HD],
                        v_ps[:].rearrange("p (h d) -> p h d", h=HL))

                def dummy_burst(S, n):
                    """Throwaway matmuls into an already-consumed score tile:
                    keeps the PE array busy across pipeline gaps so the HAM
                    clock gate stays at full rate."""
                    for _ in range(n):
                        nc.tensor.matmul(
                            S[:, 0:FW],
                            lhsT=k01_sb[0:64, 0:128],
                            rhs=k01_sb[0:64, 0:FW],
                        )

                # tiny warmup collective: absorbs the ~60us first-collective
                # CC-core boot latency long before the real AllGathers.
                nc.vector.memset(warm_sb[:], 0.0)
                nc.sync.dma_start(out=warm_in[:], in_=warm_sb[:])
                nc.gpsimd.collective_compute(
                    "AllGather",
                    mybir.AluOpType.bypass,
                    replica_groups=RG,
                    ins=[warm_in.opt()],
                    outs=[warm_out.opt()],
                )

                # prologue: only what the first score pair needs (k2d f0
                # covers k-chunks 0-3; q2d f0/f1 cover window 0); k2d f1-3
                # ride as early phase-A fillers, 2 iterations ahead of use
                qk_group(k2d_sb, wka_sb, 128, 0)
                qk_group(q2d_sb, wqa_sb, 128, 0)
                qk_group(q2d_sb, wqa_sb, 128, 1)

                # ---- attention + AllGather + local projection ----
                with tc.tile_pool(name="att_sb", bufs=1) as AS:
                    def scores_pair(w, lhs_tile, rhs_tile, kc0, kc1, Sa, Sb):
                        """Two K=64 score matmuls row-tiled (0,0)/(64,0)."""
                        q0 = w * QW
                        for j in range(QW // FW):
                            js = slice(q0 + j * FW, q0 + (j + 1) * FW)
                            ps_js = slice(j * FW, (j + 1) * FW)
                            nc.tensor.matmul(
                                Sa[:, ps_js],
                                lhsT=lhs_tile[0:64, kc0 * 128:(kc0 + 1) * 128],
                                rhs=rhs_tile[0:64, js],
                            )
                            nc.tensor.matmul(
                                Sb[:, ps_js],
                                lhsT=lhs_tile[64:128, kc1 * 128:(kc1 + 1) * 128],
                                rhs=rhs_tile[64:128, js],
                            )

                    def av_accum(A, E, kc, h, first, last):
                        for j in range(QW // FW):
                            ps_js = slice(j * FW, (j + 1) * FW)
                            nc.tensor.matmul(
                                A[:, ps_js],
                                lhsT=v_sb[:, kc, h, :],
                                rhs=E[:, ps_js],
                                start=first, stop=last,
                            )

                    def normalize(A, w, h):
                        """ag_in[w] rows h*64:.. = A[0:64] / A[64] (denom)."""
                        at = AS.tile([64, QW], F16, tag="at", bufs=3)
                        for j in range(QW // FW):
                            js = slice(j * FW, (j + 1) * FW)
                            den = AS.tile([1, FW], F32, tag="den", bufs=4)
                            bcs = AS.tile([64, FW], F32, tag="bcs", bufs=4)
                            rcp = AS.tile([64, FW], F32, tag="rcp", bufs=4)
                            nc.vector.tensor_copy(den[:], A[64:65, js])
                            nc.gpsimd.partition_broadcast(bcs[:], den[:])
                            nc.vector.reciprocal_approx_fast(rcp[:], bcs[:])
                            nc.vector.tensor_mul(at[:, js], A[0:64, js],
                                                 rcp[:])
                        nc.sync.dma_start(
                            out=ag_ins[w][h * 64:(h + 1) * 64, :], in_=at[:])

                    def all_gather(w):
                        nc.gpsimd.collective_compute(
                            "AllGather",
                            mybir.AluOpType.bypass,
                            replica_groups=RG,
                            ins=[ag_ins[w].opt()],
                            outs=[ag_outs[w].opt()],
                        )

                    def atf_dma(w, kc, j):
                        """Fetch gathered at_full chunk into SBUF."""
                        js = slice(j * FW, (j + 1) * FW)
                        nc.sync.dma_start(
                            out=atf_sb[w][:, kc, js],
                            in_=ag_outs[w][kc * 128:(kc + 1) * 128, js])

                    def proj_m(w, m):
                        """out^T[m-chunk of local c-slice, window w]."""
                        mlo = m * 128
                        msz = min(128, CHL - mlo)
                        pr = PS.tile([msz, QW], F32, tag="sc", bufs=2,
                                     padded_shape=[128, QW], name="pr")
                        for j in range(QW // FW):
                            ps_js = slice(j * FW, (j + 1) * FW)
                            for kc in range(CT):
                                nc.tensor.matmul(
                                    pr[:, ps_js],
                                    lhsT=wp_sb[:, kc, mlo:mlo + msz],
                                    rhs=atf_sb[w][:, kc, ps_js],
                                    start=(kc == 0), stop=(kc == CT - 1),
                                )
                        po = AS.tile([msz, QW], F16, tag="po", bufs=3)
                        nc.vector.tensor_scalar_add(
                            po[:], pr[:], bp_sb[0:msz, m:m + 1])
                        nc.sync.dma_start(
                            out=out_d[mlo:mlo + msz, w * QW:(w + 1) * QW],
                            in_=po[:])

                    def attn_h2(w, interleave):
                        """Head 2, adjacent-k-chunk row-tile-paired.
                        av_accums trail the scores/exp stream by one kcp so
                        the in-order PE queue never head-of-line blocks on
                        the accumulator psum slot (which waits for the
                        previous phase's normalize) at a phase seam."""
                        A2 = PS.tile([VW, QW], F32, tag="ac", bufs=2,
                                     padded_shape=[128, QW], name=f"A2_{w}")
                        pend = []
                        for kcp in range(KT // 2):
                            kc0, kc1 = 2 * kcp, 2 * kcp + 1
                            Se = PS.tile([128, QW], F32, tag="sc", bufs=2)
                            So = PS.tile([128, QW], F32, tag="sc", bufs=2)
                            scores_pair(w, k2d_sb, q2d_sb, kc0, kc1, Se, So)
                            Ee = AS.tile([128, QW], F16, tag="E", bufs=10)
                            Eo = AS.tile([128, QW], F16, tag="E", bufs=10)
                            nc.scalar.activation(Ee[:], Se[:], AF.Exp,
                                                 scale=SCALE)
                            nc.scalar.activation(Eo[:], So[:], AF.Exp,
                                                 scale=SCALE)
                            pend += [(Ee, kc0), (Eo, kc1)]
                            interleave(kcp)
                            while len(pend) > 2:
                                E, kc = pend.pop(0)
                                av_accum(A2, E, kc, 2, kc == 0, kc == KT - 1)
                        for E, kc in pend:
                            av_accum(A2, E, kc, 2, kc == 0, kc == KT - 1)
                        return A2

                    def attn_h01(w, interleave):
                        """Heads 0/1, head-row-tile-paired; accums trail by
                        two k-chunks (see attn_h2)."""
                        A0 = PS.tile([VW, QW], F32, tag="ac", bufs=2,
                                     padded_shape=[128, QW], name=f"A0_{w}")
                        A1 = PS.tile([VW, QW], F32, tag="ac", bufs=2,
                                     padded_shape=[128, QW], name=f"A1_{w}")
                        pend = []
                        for kc in range(KT):
                            S0 = PS.tile([128, QW], F32, tag="sc", bufs=2)
                            S1 = PS.tile([128, QW], F32, tag="sc", bufs=2)
                            scores_pair(w, k01_sb, q01_sb, kc, kc, S0, S1)
                            E0 = AS.tile([128, QW], F16, tag="E", bufs=10)
                            E1 = AS.tile([128, QW], F16, tag="E", bufs=10)
                            nc.scalar.activation(E0[:], S0[:], AF.Exp,
                                                 scale=SCALE)
                            nc.scalar.activation(E1[:], S1[:], AF.Exp,
                                                 scale=SCALE)
                            pend.append((E0, E1, kc))
                            interleave(kc)
                            while len(pend) > 2:
                                E0p, E1p, kcp_ = pend.pop(0)
                                av_accum(A0, E0p, kcp_, 0, kcp_ == 0,
                                         kcp_ == KT - 1)
                                av_accum(A1, E1p, kcp_, 1, kcp_ == 0,
                                         kcp_ == KT - 1)
                        for E0p, E1p, kcp_ in pend:
                            av_accum(A0, E0p, kcp_, 0, kcp_ == 0,
                                     kcp_ == KT - 1)
                            av_accum(A1, E1p, kcp_, 1, kcp_ == 0,
                                     kcp_ == KT - 1)
                        return A0, A1

                    # phase A: w0 head 2; all v chunks + phase-B q/k deps as
                    # fillers.  Accums for kc pair j flush at iteration j+1,
                    # after interleave(j+1), so v[2j]/v[2j+1] (supplied at
                    # iteration j) always precede their consumption.
                    def fillA(kcp):
                        v_group(2 * kcp)
                        v_group(2 * kcp + 1)
                        if kcp < 3:
                            # k2d f(kcp+1): consumed by score pairs at
                            # iteration 2*(kcp+1), two iterations later
                            qk_group(k2d_sb, wka_sb, 128, kcp + 1)
                        elif kcp == 3:
                            qk_group(k01_sb, wka_sb, 0, 0)
                        elif kcp == 4:
                            qk_group(q01_sb, wqa_sb, 0, 0)
                        elif kcp == 5:
                            qk_group(q01_sb, wqa_sb, 0, 1)
                    A2 = attn_h2(0, fillA)
                    # phase B: w0 heads 0/1; rest of the q/k projections
                    normalize(A2, 0, 2)
                    fillB = [
                        lambda: qk_group(k01_sb, wka_sb, 0, 1),
                        lambda: qk_group(k01_sb, wka_sb, 0, 2),
                        lambda: qk_group(k01_sb, wka_sb, 0, 3),
                        lambda: qk_group(q01_sb, wqa_sb, 0, 2),
                        lambda: qk_group(q01_sb, wqa_sb, 0, 3),
                        lambda: qk_group(q2d_sb, wqa_sb, 128, 2),
                        lambda: qk_group(q2d_sb, wqa_sb, 128, 3),
                    ]
                    A0, A1 = attn_h01(
                        0, lambda kc: fillB[kc]() if kc < len(fillB) else None)
                    # phase C: w1 head 2; AG(w0) fires here and hides under
                    # phases C/D
                    normalize(A0, 0, 0)
                    normalize(A1, 0, 1)
                    all_gather(0)
                    A2 = attn_h2(1, lambda kcp: None)
                    # phase D: w1 heads 0/1; w0 projection as fillers once
                    # AG(w0) has landed (~2/3 through the phase)
                    normalize(A2, 1, 2)

                    def fillD(kc):
                        if kc == 6:
                            for j in range(QW // FW):
                                for ct in range(CT):
                                    atf_dma(0, ct, j)
                        elif kc == 10:
                            proj_m(0, 0)
                        elif kc == 13:
                            proj_m(0, 1)
                    A0, A1 = attn_h01(1, fillD)

                    # ---- tail: normalize w1 h01, AllGather, local proj ----
                    normalize(A0, 1, 0)
                    normalize(A1, 1, 1)
                    all_gather(1)
                    # dummy burst bridges the PE across the AG wait so the
                    # HAM un-throttle is re-armed for the projection.
                    dumT = PS.tile([128, QW], F32, tag="sc", bufs=2)
                    dummy_burst(dumT, 10)
                    for j in range(QW // FW):
                        for ct in range(CT):
                            atf_dma(1, ct, j)
                    proj_m(1, 0)
                    proj_m(1, 1)
    nc.finalize()
    return nc


def get_nc():
    if "nc" not in _CACHE:
        _CACHE["nc"] = _build_nc()
    return _CACHE["nc"]


def make_in_maps(x, w_qkv, w_proj, b_proj):
    x = np.asarray(x, dtype=np.float32)
    w_qkv = np.asarray(w_qkv, dtype=np.float32)
    w_proj = np.asarray(w_proj, dtype=np.float32)
    b_proj = np.asarray(b_proj, dtype=np.float32)
    in_maps = []
    for core in range(NCORES):
        b, g = divmod(core, G)
        cs = slice(g * CHL, (g + 1) * CHL)
        wq = w_qkv[:, 0 * C:1 * C][:, cs]
        wk = w_qkv[:, 1 * C:2 * C][:, cs]
        wv = w_qkv[:, 2 * C:3 * C][:, cs]
        # [heads01 | head2 | head2-dup]
        wqa = np.concatenate([wq[:, 0:128], wq[:, 128:192], wq[:, 128:192]],
                             axis=1)
        wka = np.concatenate([wk[:, 0:128], wk[:, 128:192], wk[:, 128:192]],
                             axis=1)
        # bias for the local c_out slice, [128, 2] column-per-m-chunk
        bp = np.zeros((128, 2), dtype=np.float32)
        bp[:, 0] = b_proj[cs][0:128]
        bp[0:64, 1] = b_proj[cs][128:192]
        im = {
            "xT": np.ascontiguousarray(x[b].T, dtype=np.float16),
            "wqa": np.ascontiguousarray(wqa, dtype=np.float16),
            "wka": np.ascontiguousarray(wka, dtype=np.float16),
            "wv": np.ascontiguousarray(wv, dtype=np.float16),
            "wp": np.ascontiguousarray(w_proj[:, cs], dtype=np.float16),
            "bp": bp,
        }
        in_maps.append(im)
    return in_maps


def unshard(results):
    out = np.empty((B, N, C), dtype=np.float32)
    for b in range(B):
        outT = np.concatenate(
            [np.asarray(results[b * G + g]["out"], dtype=np.float32)
             for g in range(G)], axis=0)
        out[b] = outT.T
    return out


def kernel(x, w_qkv, w_proj, b_proj):
    from concourse.bass_utils import run_bass_kernel_spmd

    nc = get_nc()
    in_maps = make_in_maps(x, w_qkv, w_proj, b_proj)
    res = run_bass_kernel_spmd(nc, in_maps, list(range(NCORES)))
    return unshard(res.results)


# revision 7
# speedup vs baseline: 1.3072x; 1.0875x over previous
"""Multi-head attention (B=2, N=2048, C=768, H=12) on 8 trn2 cores.

Sharding: core i handles batch b = i//4 and head-group g = i%4 (3 heads).
All device data is fp16 (tolerance 2e-2 allows it); matmul accumulation
stays fp32 in PSUM.

Per-core pipeline:
  1. QKV^T projection from host-pre-transposed xT [C, N]:
       q01/k01  [128, N]: heads 0,1 d-major (h0 at partitions 0:64, h1 at
                64:128) -> natural row-tile pairing for the score matmuls.
       q2d/k2d  [128, N]: head 2 duplicated in both partition halves so its
                score matmuls can be row-tile paired across adjacent k-chunks.
       v        [N, 65] per (k-chunk, head): cols 0:64 = v, col 64 = ones
                (softmax denominator trick).
  2. Scores transposed: S^T[k, q] = k_h^T-chunk.T @ q_h. Heads 0/1 (and for
     head 2, adjacent k-chunks) issue as K=64 matmuls at tile_position
     (0,0)/(64,0) -> they stream concurrently in the PE array.
  3. exp via ScalarE (the kernel's throughput floor: ~96 activations of
     [128,1024]); output fp16 to SBUF.
  4. attn@V with lhsT = [v | 1]: psum rows 0:64 = unnormalized attn_out^T,
     row 64 = denominators.  Normalize: gpsimd partition-broadcast of the
     denominator row, DVE reciprocal_approx_fast, DVE multiply -> fp16.
  5. Output exchange via per-window, per-head-group AllGathers of the
     NORMALIZED attention output (4x less data than ReduceScattering
     projection partials).  Each window w sends two chunks: h2 [64, QW]
     (ready a full phase before h01) and h01 [128, QW].  w_proj rows are
     host-permuted to the gathered order ([4 groups' h2 | 4 groups'
     h01-pairs]) so each core computes its c_out slice of the projection
     locally at full K=128 efficiency, bias folded into the psum->sbuf
     copy (DVE tensor_scalar add).  AG(w0-h2)/AG(w0-h01) hide under
     phases B/C, AG(w1-h2) under phase D; only AG(w1-h01) (~18us) plus a
     short local projection is tail-exposed.

Scheduling: attention is one software pipeline in 4 phases (w0-h2,
w0-h01, w1-h2, w1-h01).  QKV projection groups and v-projection chunks
interleave into phases A/B as PE gap fillers (the PE otherwise idles
under the ScalarE exp pace and the HAM clock gate then halves its
clock).  attn@V accums trail the score/exp stream by two k-chunks so
the in-order PE queue never head-of-line blocks on an accumulator psum
slot that waits for the previous phase's normalize.  A tiny warmup
collective at kernel start absorbs the ~60us first-collective CC-core
boot latency.  Input DMAs are f-major so the first projection groups
unblock after ~1MB instead of the full 3.1MB xT load.
"""

import numpy as np

B, N, C, H, HD = 2, 2048, 768, 12, 64
G = 4              # tensor-parallel head groups
HL = H // G        # 3 heads per core
CHL = HL * HD      # 192 local channels
SCALE = HD ** -0.5
NCORES = 8
CT = C // 128      # 6 contraction chunks
FW = 512           # matmul free width (psum bank)
QW = 1024          # q window width
NWIN = N // QW     # 2 windows
KT = N // 128      # 16 k chunks
VW = HD + 1        # v tile cols: 64 v + 1 ones

_CACHE = {}


def _build_nc():
    import concourse.bass as bass
    import concourse.bacc as bacc
    import concourse.tile as tile
    import concourse.mybir as mybir

    F32 = mybir.dt.float32
    F16 = mybir.dt.float16
    AF = mybir.ActivationFunctionType
    RG = [[0, 1, 2, 3], [4, 5, 6, 7]]

    nc = bacc.Bacc(num_devices=NCORES)
    xT_d = nc.declare_dram_parameter("xT", [C, N], F16, isOutput=False)
    wqa_d = nc.declare_dram_parameter("wqa", [C, 256], F16, isOutput=False)
    wka_d = nc.declare_dram_parameter("wka", [C, 256], F16, isOutput=False)
    wv_d = nc.declare_dram_parameter("wv", [C, CHL], F16, isOutput=False)
    wp_d = nc.declare_dram_parameter("wp", [C, CHL], F16, isOutput=False)
    bp_d = nc.declare_dram_parameter("bp", [128, 2], F32, isOutput=False)
    out_d = nc.declare_dram_parameter("out", [CHL, N], F16, isOutput=True)

    with tile.TileContext(nc) as tc:
        with tc.tile_pool(name="dram", bufs=1, space="DRAM") as dram:
            # per window: h2 chunk [64, QW] and h01 chunk [128, QW]
            ag2_ins = [dram.tile([64, QW], F16, name=f"ag2_in{w}")
                       for w in range(NWIN)]
            ag2_outs = [dram.tile([256, QW], F16, name=f"ag2_out{w}")
                        for w in range(NWIN)]
            ag01_ins = [dram.tile([128, QW], F16, name=f"ag01_in{w}")
                        for w in range(NWIN)]
            ag01_outs = [dram.tile([512, QW], F16, name=f"ag01_out{w}")
                         for w in range(NWIN)]
            warm_in = dram.tile([8, 128], F16, name="warm_in")
            warm_out = dram.tile([32, 128], F16, name="warm_out")

            with tc.tile_pool(name="sb", bufs=1) as P, \
                    tc.tile_pool(name="ps", bufs=1, space="PSUM") as PS:
                # ---- input DMAs: consolidated (one instruction per
                # tensor / xT half), weights on the scalar HWDGE queue so
                # they stream in parallel with xT on the sync queue.  The
                # prologue needs only wka/wqa + xT cols 0:1024. ----
                wka_sb = P.tile([128, CT, 256], F16)
                wqa_sb = P.tile([128, CT, 256], F16)
                xT_sb = P.tile([128, CT, N], F16)
                nc.scalar.dma_start(
                    out=wka_sb[:],
                    in_=wka_d[:, :].rearrange("(c p) m -> p c m", p=128))
                nc.sync.dma_start(
                    out=xT_sb[:, :, 0:QW],
                    in_=xT_d[:, 0:QW].rearrange("(c p) n -> p c n", p=128))
                nc.scalar.dma_start(
                    out=wqa_sb[:],
                    in_=wqa_d[:, :].rearrange("(c p) m -> p c m", p=128))
                wv_sb = P.tile([128, CT, CHL], F16)
                nc.scalar.dma_start(
                    out=wv_sb[:],
                    in_=wv_d[:, :].rearrange("(c p) m -> p c m", p=128))
                nc.sync.dma_start(
                    out=xT_sb[:, :, QW:N],
                    in_=xT_d[:, QW:N].rearrange("(c p) n -> p c n", p=128))
                # w_proj column-slice [C, 192], rows host-permuted to the
                # gathered order; row chunks ct
                wp_sb = P.tile([128, CT, CHL], F16)
                nc.scalar.dma_start(
                    out=wp_sb[:],
                    in_=wp_d[:, :].rearrange("(c p) m -> p c m", p=128))
                bp_sb = P.tile([128, 2], F32)
                nc.scalar.dma_start(out=bp_sb[:], in_=bp_d[:, :])
                warm_sb = P.tile([8, 128], F16)

                # ---- persistent QKV results ----
                k01_sb = P.tile([128, N], F16)
                q01_sb = P.tile([128, N], F16)
                k2d_sb = P.tile([128, N], F16)
                q2d_sb = P.tile([128, N], F16)
                # [n, kt, h, VW]: cols 0:64 = v, col 64 = ones
                v_sb = P.tile([128, KT, HL, VW], F16)
                nc.vector.memset(v_sb[:, :, :, HD:VW], 1.0)
                # gathered attention output, [c_in chunk, kc, q]
                atf_sb = [P.tile([128, CT, QW], F16, name=f"atf{w}")
                          for w in range(NWIN)]

                # ---- QKV projection emitters (interleaved into attention
                # as PE gap-fillers: keeps TensorE dense so HAM stays at
                # full clock) ----
                # psum ring "sc": [128,1024] slots (2 banks) x2 -> qk/v
                # projection psum, score tiles, and proj psum.
                # psum ring "ac": [128,1024] slots x2 -> attention
                # accumulators only.
                def qk_group(dst, wsb, mlo, f):
                    qk_ps = PS.tile([128, FW], F32, tag="sc", bufs=2,
                                    padded_shape=[128, QW], name="qk_ps")
                    for ct in range(CT):
                        nc.tensor.matmul(
                            qk_ps[:],
                            lhsT=wsb[:, ct, mlo:mlo + 128],
                            rhs=xT_sb[:, ct, f * FW:(f + 1) * FW],
                            start=(ct == 0), stop=(ct == CT - 1),
                        )
                    nc.vector.tensor_copy(
                        dst[:, f * FW:(f + 1) * FW], qk_ps[:])

                def v_group(nt):
                    v_ps = PS.tile([128, CHL], F32, tag="sc", bufs=2,
                                   padded_shape=[128, QW], name="v_ps")
                    for ct in range(CT):
                        nc.tensor.matmul(
                            v_ps[:],
                            lhsT=xT_sb[:, ct, nt * 128:(nt + 1) * 128],
                            rhs=wv_sb[:, ct, :],
                            start=(ct == 0), stop=(ct == CT - 1),
                        )
                    nc.vector.tensor_copy(
                        v_sb[:, nt, :, 0:HD],
                        v_ps[:].rearrange("p (h d) -> p h d", h=HL))

                def dummy_burst(S, n):
                    """Throwaway matmuls into an already-consumed score tile:
                    keeps the PE array busy across pipeline gaps so the HAM
                    clock gate stays at full rate."""
                    for _ in range(n):
                        nc.tensor.matmul(
                            S[:, 0:FW],
                            lhsT=k01_sb[0:64, 0:128],
                            rhs=k01_sb[0:64, 0:FW],
                        )

                # tiny warmup collective: absorbs the ~60us first-collective
                # CC-core boot latency long before the real AllGathers.
                nc.vector.memset(warm_sb[:], 0.0)
                nc.sync.dma_start(out=warm_in[:], in_=warm_sb[:])
                nc.gpsimd.collective_compute(
                    "AllGather",
                    mybir.AluOpType.bypass,
                    replica_groups=RG,
                    ins=[warm_in.opt()],
                    outs=[warm_out.opt()],
                )

                # prologue: only what the first score pair needs (k2d f0
                # covers k-chunks 0-3; q2d f0/f1 cover window 0); k2d f1-3
                # ride as early phase-A fillers, 2 iterations ahead of use
                qk_group(k2d_sb, wka_sb, 128, 0)
                qk_group(q2d_sb, wqa_sb, 128, 0)
                qk_group(q2d_sb, wqa_sb, 128, 1)

                # ---- attention + AllGather + local projection ----
                with tc.tile_pool(name="att_sb", bufs=1) as AS:
                    def scores_pair(w, lhs_tile, rhs_tile, kc0, kc1, Sa, Sb):
                        """Two K=64 score matmuls row-tiled (0,0)/(64,0)."""
                        q0 = w * QW
                        for j in range(QW // FW):
                            js = slice(q0 + j * FW, q0 + (j + 1) * FW)
                            ps_js = slice(j * FW, (j + 1) * FW)
                            nc.tensor.matmul(
                                Sa[:, ps_js],
                                lhsT=lhs_tile[0:64, kc0 * 128:(kc0 + 1) * 128],
                                rhs=rhs_tile[0:64, js],
                            )
                            nc.tensor.matmul(
                                Sb[:, ps_js],
                                lhsT=lhs_tile[64:128, kc1 * 128:(kc1 + 1) * 128],
                                rhs=rhs_tile[64:128, js],
                            )

                    def av_accum(A, E, kc, h, first, last):
                        for j in range(QW // FW):
                            ps_js = slice(j * FW, (j + 1) * FW)
                            nc.tensor.matmul(
                                A[:, ps_js],
                                lhsT=v_sb[:, kc, h, :],
                                rhs=E[:, ps_js],
                                start=first, stop=last,
                            )

                    def normalize(A, w, h):
                        """ag in rows for head h = A[0:64] / A[64] (denom)."""
                        at = AS.tile([64, QW], F16, tag="at", bufs=3)
                        for j in range(QW // FW):
                            js = slice(j * FW, (j + 1) * FW)
                            den = AS.tile([1, FW], F32, tag="den", bufs=4)
                            bcs = AS.tile([64, FW], F32, tag="bcs", bufs=4)
                            rcp = AS.tile([64, FW], F32, tag="rcp", bufs=4)
                            nc.vector.tensor_copy(den[:], A[64:65, js])
                            nc.gpsimd.partition_broadcast(bcs[:], den[:])
                            nc.vector.reciprocal_approx_fast(rcp[:], bcs[:])
                            nc.vector.tensor_mul(at[:, js], A[0:64, js],
                                                 rcp[:])
                        if h == 2:
                            nc.sync.dma_start(out=ag2_ins[w][:, :], in_=at[:])
                        else:
                            nc.sync.dma_start(
                                out=ag01_ins[w][h * 64:(h + 1) * 64, :],
                                in_=at[:])

                    def all_gather(w, part):
                        ins = ag2_ins if part == 2 else ag01_ins
                        outs = ag2_outs if part == 2 else ag01_outs
                        nc.gpsimd.collective_compute(
                            "AllGather",
                            mybir.AluOpType.bypass,
                            replica_groups=RG,
                            ins=[ins[w].opt()],
                            outs=[outs[w].opt()],
                        )

                    def atf_dma(w, j):
                        """Fetch gathered at_full half-window into SBUF.
                        c_in chunks 0:2 come from the h2 gather, 2:6 from
                        the h01 gather (w_proj rows are permuted to match).
                        """
                        js = slice(j * FW, (j + 1) * FW)
                        nc.sync.dma_start(
                            out=atf_sb[w][:, 0:2, js],
                            in_=ag2_outs[w][:, js].rearrange(
                                "(c p) n -> p c n", p=128))
                        nc.sync.dma_start(
                            out=atf_sb[w][:, 2:CT, js],
                            in_=ag01_outs[w][:, js].rearrange(
                                "(c p) n -> p c n", p=128))

                    def proj_m(w, m):
                        """out^T[m-chunk of local c-slice, window w]."""
                        mlo = m * 128
                        msz = min(128, CHL - mlo)
                        pr = PS.tile([msz, QW], F32, tag="sc", bufs=2,
                                     padded_shape=[128, QW], name="pr")
                        for j in range(QW // FW):
                            ps_js = slice(j * FW, (j + 1) * FW)
                            for kc in range(CT):
                                nc.tensor.matmul(
                                    pr[:, ps_js],
                                    lhsT=wp_sb[:, kc, mlo:mlo + msz],
                                    rhs=atf_sb[w][:, kc, ps_js],
                                    start=(kc == 0), stop=(kc == CT - 1),
                                )
                        po = AS.tile([msz, QW], F16, tag="po", bufs=3)
                        nc.vector.tensor_scalar_add(
                            po[:], pr[:], bp_sb[0:msz, m:m + 1])
                        nc.sync.dma_start(
                            out=out_d[mlo:mlo + msz, w * QW:(w + 1) * QW],
                            in_=po[:])

                    def attn_h2(w, interleave):
                        """Head 2, adjacent-k-chunk row-tile-paired.
                        av_accums trail the scores/exp stream by one kcp so
                        the in-order PE queue never head-of-line blocks on
                        the accumulator psum slot (which waits for the
                        previous phase's normalize) at a phase seam."""
                        A2 = PS.tile([VW, QW], F32, tag="ac", bufs=2,
                                     padded_shape=[128, QW], name=f"A2_{w}")
                        pend = []
                        for kcp in range(KT // 2):
                            kc0, kc1 = 2 * kcp, 2 * kcp + 1
                            Se = PS.tile([128, QW], F32, tag="sc", bufs=2)
                            So = PS.tile([128, QW], F32, tag="sc", bufs=2)
                            scores_pair(w, k2d_sb, q2d_sb, kc0, kc1, Se, So)
                            Ee = AS.tile([128, QW], F16, tag="E", bufs=10)
                            Eo = AS.tile([128, QW], F16, tag="E", bufs=10)
                            nc.scalar.activation(Ee[:], Se[:], AF.Exp,
                                                 scale=SCALE)
                            nc.scalar.activation(Eo[:], So[:], AF.Exp,
                                                 scale=SCALE)
                            pend += [(Ee, kc0), (Eo, kc1)]
                            interleave(kcp)
                            while len(pend) > 2:
                                E, kc = pend.pop(0)
                                av_accum(A2, E, kc, 2, kc == 0, kc == KT - 1)
                        for E, kc in pend:
                            av_accum(A2, E, kc, 2, kc == 0, kc == KT - 1)
                        return A2

                    def attn_h01(w, interleave):
                        """Heads 0/1, head-row-tile-paired; accums trail by
                        two k-chunks (see attn_h2)."""
                        A0 = PS.tile([VW, QW], F32, tag="ac", bufs=2,
                                     padded_shape=[128, QW], name=f"A0_{w}")
                        A1 = PS.tile([VW, QW], F32, tag="ac", bufs=2,
                                     padded_shape=[128, QW], name=f"A1_{w}")
                        pend = []
                        for kc in range(KT):
                            S0 = PS.tile([128, QW], F32, tag="sc", bufs=2)
                            S1 = PS.tile([128, QW], F32, tag="sc", bufs=2)
                            scores_pair(w, k01_sb, q01_sb, kc, kc, S0, S1)
                            E0 = AS.tile([128, QW], F16, tag="E", bufs=10)
                            E1 = AS.tile([128, QW], F16, tag="E", bufs=10)
                            nc.scalar.activation(E0[:], S0[:], AF.Exp,
                                                 scale=SCALE)
                            nc.scalar.activation(E1[:], S1[:], AF.Exp,
                                                 scale=SCALE)
                            pend.append((E0, E1, kc))
                            interleave(kc)
                            while len(pend) > 2:
                                E0p, E1p, kcp_ = pend.pop(0)
                                av_accum(A0, E0p, kcp_, 0, kcp_ == 0,
                                         kcp_ == KT - 1)
                                av_accum(A1, E1p, kcp_, 1, kcp_ == 0,
                                         kcp_ == KT - 1)
                        for E0p, E1p, kcp_ in pend:
                            av_accum(A0, E0p, kcp_, 0, kcp_ == 0,
                                     kcp_ == KT - 1)
                            av_accum(A1, E1p, kcp_, 1, kcp_ == 0,
                                     kcp_ == KT - 1)
                        return A0, A1

                    # phase A: w0 head 2; all v chunks + phase-B q/k deps as
                    # fillers.  Accums for kc pair j flush at iteration j+1,
                    # after interleave(j+1), so v[2j]/v[2j+1] (supplied at
                    # iteration j) always precede their consumption.
                    def fillA(kcp):
                        v_group(2 * kcp)
                        v_group(2 * kcp + 1)
                        if kcp < 3:
                            # k2d f(kcp+1): consumed by score pairs at
                            # iteration 2*(kcp+1), two iterations later
                            qk_group(k2d_sb, wka_sb, 128, kcp + 1)
                        elif kcp == 3:
                            qk_group(k01_sb, wka_sb, 0, 0)
                        elif kcp == 4:
                            qk_group(q01_sb, wqa_sb, 0, 0)
                        elif kcp == 5:
                            qk_group(q01_sb, wqa_sb, 0, 1)
                    A2 = attn_h2(0, fillA)
                    # phase B: w0 heads 0/1; rest of the q/k projections.
                    # AG(w0-h2) fires at phase B start and hides here.
                    normalize(A2, 0, 2)
                    all_gather(0, 2)
                    fillB = [
                        lambda: qk_group(k01_sb, wka_sb, 0, 1),
                        lambda: qk_group(k01_sb, wka_sb, 0, 2),
                        lambda: qk_group(k01_sb, wka_sb, 0, 3),
                        lambda: qk_group(q01_sb, wqa_sb, 0, 2),
                        lambda: qk_group(q01_sb, wqa_sb, 0, 3),
                        lambda: qk_group(q2d_sb, wqa_sb, 128, 2),
                        lambda: qk_group(q2d_sb, wqa_sb, 128, 3),
                    ]
                    A0, A1 = attn_h01(
                        0, lambda kc: fillB[kc]() if kc < len(fillB) else None)
                    # phase C: w1 head 2; AG(w0-h01) fires here and hides
                    # under phase C
                    normalize(A0, 0, 0)
                    normalize(A1, 0, 1)
                    all_gather(0, 0)
                    A2 = attn_h2(1, lambda kcp: None)
                    # phase D: w1 heads 0/1; AG(w1-h2) hides here and the
                    # w0 projection runs as fillers once AG(w0) has landed
                    normalize(A2, 1, 2)
                    all_gather(1, 2)

                    def fillD(kc):
                        if kc == 2:
                            for j in range(QW // FW):
                                atf_dma(0, j)
                        elif kc == 5:
                            proj_m(0, 0)
                        elif kc == 9:
                            proj_m(0, 1)
                    A0, A1 = attn_h01(1, fillD)

                    # ---- tail: normalize w1 h01, AllGather, local proj ----
                    normalize(A0, 1, 0)
                    normalize(A1, 1, 1)
                    all_gather(1, 0)
                    # dummy burst bridges the PE across the AG wait so the
                    # HAM un-throttle is re-armed for the projection.
                    dumT = PS.tile([128, QW], F32, tag="sc", bufs=2)
                    dummy_burst(dumT, 6)
                    for j in range(QW // FW):
                        atf_dma(1, j)
                    proj_m(1, 0)
                    proj_m(1, 1)
    nc.finalize()
    return nc


def get_nc():
    if "nc" not in _CACHE:
        _CACHE["nc"] = _build_nc()
    return _CACHE["nc"]


def make_in_maps(x, w_qkv, w_proj, b_proj):
    x = np.asarray(x, dtype=np.float32)
    w_qkv = np.asarray(w_qkv, dtype=np.float32)
    w_proj = np.asarray(w_proj, dtype=np.float32)
    b_proj = np.asarray(b_proj, dtype=np.float32)
    in_maps = []
    for core in range(NCORES):
        b, g = divmod(core, G)
        cs = slice(g * CHL, (g + 1) * CHL)
        wq = w_qkv[:, 0 * C:1 * C][:, cs]
        wk = w_qkv[:, 1 * C:2 * C][:, cs]
        wv = w_qkv[:, 2 * C:3 * C][:, cs]
        # [heads01 | head2 | head2-dup]
        wqa = np.concatenate([wq[:, 0:128], wq[:, 128:192], wq[:, 128:192]],
                             axis=1)
        wka = np.concatenate([wk[:, 0:128], wk[:, 128:192], wk[:, 128:192]],
                             axis=1)
        # bias for the local c_out slice, [128, 2] column-per-m-chunk
        bp = np.zeros((128, 2), dtype=np.float32)
        bp[:, 0] = b_proj[cs][0:128]
        bp[0:64, 1] = b_proj[cs][128:192]
        # w_proj rows permuted to the gathered at_full order:
        # [4 groups' h2 | 4 groups' (h0, h1)]
        head_order = [2, 5, 8, 11, 0, 1, 3, 4, 6, 7, 9, 10]
        row_perm = np.concatenate(
            [np.arange(h * HD, (h + 1) * HD) for h in head_order])
        im = {
            "xT": np.ascontiguousarray(x[b].T, dtype=np.float16),
            "wqa": np.ascontiguousarray(wqa, dtype=np.float16),
            "wka": np.ascontiguousarray(wka, dtype=np.float16),
            "wv": np.ascontiguousarray(wv, dtype=np.float16),
            "wp": np.ascontiguousarray(w_proj[row_perm][:, cs],
                                       dtype=np.float16),
            "bp": bp,
        }
        in_maps.append(im)
    return in_maps


def unshard(results):
    out = np.empty((B, N, C), dtype=np.float32)
    for b in range(B):
        outT = np.concatenate(
            [np.asarray(results[b * G + g]["out"], dtype=np.float32)
             for g in range(G)], axis=0)
        out[b] = outT.T
    return out


def kernel(x, w_qkv, w_proj, b_proj):
    from concourse.bass_utils import run_bass_kernel_spmd

    nc = get_nc()
    in_maps = make_in_maps(x, w_qkv, w_proj, b_proj)
    res = run_bass_kernel_spmd(nc, in_maps, list(range(NCORES)))
    return unshard(res.results)


# revision 14
# speedup vs baseline: 1.3422x; 1.0268x over previous
"""Multi-head attention (B=2, N=2048, C=768, H=12) on 8 trn2 cores.

Sharding: core i handles batch b = i//4 and head-group g = i%4 (3 heads).
All device data is fp16 (tolerance 2e-2 allows it); matmul accumulation
stays fp32 in PSUM.

Per-core pipeline:
  1. QKV^T projection from host-pre-transposed xT [C, N]:
       q01/k01  [128, N]: heads 0,1 d-major (h0 at partitions 0:64, h1 at
                64:128) -> natural row-tile pairing for the score matmuls.
       q2d/k2d  [128, N]: head 2 duplicated in both partition halves so its
                score matmuls can be row-tile paired across adjacent k-chunks.
       v        [N, 65] per (k-chunk, head): cols 0:64 = v, col 64 = ones
                (softmax denominator trick).
  2. Scores transposed: S^T[k, q] = k_h^T-chunk.T @ q_h. Heads 0/1 (and for
     head 2, adjacent k-chunks) issue as K=64 matmuls at tile_position
     (0,0)/(64,0) -> they stream concurrently in the PE array.
  3. exp via ScalarE (the kernel's throughput floor: ~96 activations of
     [128,1024]); output fp16 to SBUF.
  4. attn@V with lhsT = [v | 1]: psum rows 0:64 = unnormalized attn_out^T,
     row 64 = denominators.  Normalize: gpsimd partition-broadcast of the
     denominator row, DVE reciprocal_approx_fast, DVE multiply -> fp16.
  5. Output exchange via per-window, per-head-group AllGathers of the
     NORMALIZED attention output (4x less data than ReduceScattering
     projection partials).  Each window w sends two chunks: h2 [64, QW]
     (ready a full phase before h01) and h01 [128, QW].  w_proj rows are
     host-permuted to the gathered order ([4 groups' h2 | 4 groups'
     h01-pairs]) so each core computes its c_out slice of the projection
     locally at full K=128 efficiency, bias folded into the psum->sbuf
     copy (DVE tensor_scalar add).  AG(w0-h2)/AG(w0-h01) hide under
     phases B/C, AG(w1-h2) under phase D; only AG(w1-h01) (~18us) plus a
     short local projection is tail-exposed.

Scheduling: attention is one software pipeline in 4 phases (w0-h2,
w0-h01, w1-h2, w1-h01).  QKV projection groups and v-projection chunks
interleave into phases A/B as PE gap fillers (the PE otherwise idles
under the ScalarE exp pace and the HAM clock gate then halves its
clock).  attn@V accums trail the score/exp stream by two k-chunks so
the in-order PE queue never head-of-line blocks on an accumulator psum
slot that waits for the previous phase's normalize.  A tiny warmup
collective at kernel start absorbs the ~60us first-collective CC-core
boot latency.  Input DMAs are f-major so the first projection groups
unblock after ~1MB instead of the full 3.1MB xT load.
"""

import numpy as np

B, N, C, H, HD = 2, 2048, 768, 12, 64
G = 4              # tensor-parallel head groups
HL = H // G        # 3 heads per core
CHL = HL * HD      # 192 local channels
SCALE = HD ** -0.5
NCORES = 8
CT = C // 128      # 6 contraction chunks
FW = 512           # matmul free width (psum bank)
QW = 1024          # q window width
NWIN = N // QW     # 2 windows
KT = N // 128      # 16 k chunks
VW = HD + 1        # v tile cols: 64 v + 1 ones

_CACHE = {}


def _build_nc():
    import concourse.bass as bass
    import concourse.bacc as bacc
    import concourse.tile as tile
    import concourse.mybir as mybir

    F32 = mybir.dt.float32
    F16 = mybir.dt.float16
    AF = mybir.ActivationFunctionType
    RG = [[0, 1, 2, 3], [4, 5, 6, 7]]

    nc = bacc.Bacc(num_devices=NCORES)
    xT_d = nc.declare_dram_parameter("xT", [C, N], F16, isOutput=False)
    wqa_d = nc.declare_dram_parameter("wqa", [C, 256], F16, isOutput=False)
    wka_d = nc.declare_dram_parameter("wka", [C, 256], F16, isOutput=False)
    wv_d = nc.declare_dram_parameter("wv", [C, CHL], F16, isOutput=False)
    wp_d = nc.declare_dram_parameter("wp", [C, CHL], F16, isOutput=False)
    bp_d = nc.declare_dram_parameter("bp", [128, 2], F32, isOutput=False)
    out_d = nc.declare_dram_parameter("out", [CHL, N], F16, isOutput=True)

    with tile.TileContext(nc) as tc:
        with tc.tile_pool(name="dram", bufs=1, space="DRAM") as dram:
            # per window: h2 chunk [64, QW] and h01 chunk [128, QW]
            ag2_ins = [dram.tile([64, QW], F16, name=f"ag2_in{w}")
                       for w in range(NWIN)]
            ag2_outs = [dram.tile([256, QW], F16, name=f"ag2_out{w}")
                        for w in range(NWIN)]
            ag01_ins = [dram.tile([128, QW], F16, name=f"ag01_in{w}")
                        for w in range(NWIN)]
            ag01_outs = [dram.tile([512, QW], F16, name=f"ag01_out{w}")
                         for w in range(NWIN)]
            warm_in = dram.tile([8, 128], F16, name="warm_in")
            warm_out = dram.tile([32, 128], F16, name="warm_out")

            with tc.tile_pool(name="sb", bufs=1) as P, \
                    tc.tile_pool(name="ps", bufs=1, space="PSUM") as PS:
                # ---- input DMAs: consolidated (one instruction per
                # tensor / xT half), weights on the scalar HWDGE queue so
                # they stream in parallel with xT on the sync queue.  The
                # prologue needs only wka/wqa + xT cols 0:1024. ----
                wka_sb = P.tile([128, CT, 256], F16)
                wqa_sb = P.tile([128, CT, 256], F16)
                xT_sb = P.tile([128, CT, N], F16)
                nc.scalar.dma_start(
                    out=wka_sb[:],
                    in_=wka_d[:, :].rearrange("(c p) m -> p c m", p=128))
                nc.sync.dma_start(
                    out=xT_sb[:, :, 0:QW],
                    in_=xT_d[:, 0:QW].rearrange("(c p) n -> p c n", p=128))
                nc.scalar.dma_start(
                    out=wqa_sb[:],
                    in_=wqa_d[:, :].rearrange("(c p) m -> p c m", p=128))
                wv_sb = P.tile([128, CT, CHL], F16)
                nc.scalar.dma_start(
                    out=wv_sb[:],
                    in_=wv_d[:, :].rearrange("(c p) m -> p c m", p=128))
                nc.sync.dma_start(
                    out=xT_sb[:, :, QW:N],
                    in_=xT_d[:, QW:N].rearrange("(c p) n -> p c n", p=128))
                # w_proj column-slice [C, 192], rows host-permuted to the
                # gathered order; row chunks ct.  On the sync queue (not
                # scalar): a long transfer on the scalar HWDGE queue would
                # block phase A's exps behind it (strict FIFO).
                wp_sb = P.tile([128, CT, CHL], F16)
                nc.sync.dma_start(
                    out=wp_sb[:],
                    in_=wp_d[:, :].rearrange("(c p) m -> p c m", p=128))
                bp_sb = P.tile([128, 2], F32)
                nc.sync.dma_start(out=bp_sb[:], in_=bp_d[:, :])
                warm_sb = P.tile([8, 128], F16)
                ones64 = P.tile([1, HD], F16)
                nc.vector.memset(ones64[:], 1.0)

                # ---- persistent QKV results ----
                k01_sb = P.tile([128, N], F16)
                q01_sb = P.tile([128, N], F16)
                k2d_sb = P.tile([128, N], F16)
                q2d_sb = P.tile([128, N], F16)
                # [n, kt, h, VW]: cols 0:64 = v, col 64 = ones
                v_sb = P.tile([128, KT, HL, VW], F16)
                nc.vector.memset(v_sb[:, :, :, HD:VW], 1.0)
                # gathered attention output, [c_in chunk, kc, q]
                atf_sb = [P.tile([128, CT, QW], F16, name=f"atf{w}")
                          for w in range(NWIN)]

                # ---- QKV projection emitters (interleaved into attention
                # as PE gap-fillers: keeps TensorE dense so HAM stays at
                # full clock) ----
                # psum ring "sc": [128,1024] slots (2 banks) x2 -> qk/v
                # projection psum, score tiles, and proj psum.
                # psum ring "ac": [128,1024] slots x2 -> attention
                # accumulators only.
                def qk_group(dst, wsb, mlo, f):
                    qk_ps = PS.tile([128, FW], F32, tag="sc", bufs=2,
                                    padded_shape=[128, QW], name="qk_ps")
                    for ct in range(CT):
                        nc.tensor.matmul(
                            qk_ps[:],
                            lhsT=wsb[:, ct, mlo:mlo + 128],
                            rhs=xT_sb[:, ct, f * FW:(f + 1) * FW],
                            start=(ct == 0), stop=(ct == CT - 1),
                        )
                    nc.vector.tensor_copy(
                        dst[:, f * FW:(f + 1) * FW], qk_ps[:])

                def v_group(nt):
                    v_ps = PS.tile([128, CHL], F32, tag="sc", bufs=2,
                                   padded_shape=[128, QW], name="v_ps")
                    for ct in range(CT):
                        nc.tensor.matmul(
                            v_ps[:],
                            lhsT=xT_sb[:, ct, nt * 128:(nt + 1) * 128],
                            rhs=wv_sb[:, ct, :],
                            start=(ct == 0), stop=(ct == CT - 1),
                        )
                    nc.vector.tensor_copy(
                        v_sb[:, nt, :, 0:HD],
                        v_ps[:].rearrange("p (h d) -> p h d", h=HL))

                def dummy_burst(S, n):
                    """Throwaway matmuls into an already-consumed score tile:
                    keeps the PE array busy across pipeline gaps so the HAM
                    clock gate stays at full rate."""
                    for _ in range(n):
                        nc.tensor.matmul(
                            S[:, 0:FW],
                            lhsT=k01_sb[0:64, 0:128],
                            rhs=k01_sb[0:64, 0:FW],
                        )

                # tiny warmup collective: absorbs the ~60us first-collective
                # CC-core boot latency long before the real AllGathers.
                nc.vector.memset(warm_sb[:], 0.0)
                nc.sync.dma_start(out=warm_in[:], in_=warm_sb[:])
                nc.gpsimd.collective_compute(
                    "AllGather",
                    mybir.AluOpType.bypass,
                    replica_groups=RG,
                    ins=[warm_in.opt()],
                    outs=[warm_out.opt()],
                )

                # prologue: only what the first score pair needs (k2d f0
                # covers k-chunks 0-3; q2d f0/f1 cover window 0); k2d f1-3
                # ride as early phase-A fillers, 2 iterations ahead of use
                qk_group(k2d_sb, wka_sb, 128, 0)
                qk_group(q2d_sb, wqa_sb, 128, 0)
                qk_group(q2d_sb, wqa_sb, 128, 1)

                # ---- attention + AllGather + local projection ----
                with tc.tile_pool(name="att_sb", bufs=1) as AS:
                    def scores_pair(w, lhs_tile, rhs_tile, kc0, kc1, Sa, Sb):
                        """Two K=64 score matmuls row-tiled (0,0)/(64,0)."""
                        q0 = w * QW
                        for j in range(QW // FW):
                            js = slice(q0 + j * FW, q0 + (j + 1) * FW)
                            ps_js = slice(j * FW, (j + 1) * FW)
                            nc.tensor.matmul(
                                Sa[:, ps_js],
                                lhsT=lhs_tile[0:64, kc0 * 128:(kc0 + 1) * 128],
                                rhs=rhs_tile[0:64, js],
                            )
                            nc.tensor.matmul(
                                Sb[:, ps_js],
                                lhsT=lhs_tile[64:128, kc1 * 128:(kc1 + 1) * 128],
                                rhs=rhs_tile[64:128, js],
                            )

                    def av_accum(A, E, kc, h, first, last):
                        for j in range(QW // FW):
                            ps_js = slice(j * FW, (j + 1) * FW)
                            nc.tensor.matmul(
                                A[:, ps_js],
                                lhsT=v_sb[:, kc, h, :],
                                rhs=E[:, ps_js],
                                start=first, stop=last,
                            )

                    def normalize(A, w, h):
                        """ag in rows for head h = A[0:64] / A[64] (denom)."""
                        at = AS.tile([64, QW], F16, tag="at", bufs=3)
                        for j in range(QW // FW):
                            js = slice(j * FW, (j + 1) * FW)
                            den = AS.tile([1, FW], F32, tag="den", bufs=4)
                            bcs = AS.tile([64, FW], F32, tag="bcs", bufs=4)
                            rcp = AS.tile([64, FW], F32, tag="rcp", bufs=4)
                            nc.vector.tensor_copy(den[:], A[64:65, js])
                            nc.gpsimd.partition_broadcast(bcs[:], den[:])
                            nc.vector.reciprocal_approx_fast(rcp[:], bcs[:])
                            nc.vector.tensor_mul(at[:, js], A[0:64, js],
                                                 rcp[:])
                        if h == 2:
                            nc.sync.dma_start(out=ag2_ins[w][:, :], in_=at[:])
                        else:
                            nc.sync.dma_start(
                                out=ag01_ins[w][h * 64:(h + 1) * 64, :],
                                in_=at[:])

                    def all_gather(w, part):
                        ins = ag2_ins if part == 2 else ag01_ins
                        outs = ag2_outs if part == 2 else ag01_outs
                        nc.gpsimd.collective_compute(
                            "AllGather",
                            mybir.AluOpType.bypass,
                            replica_groups=RG,
                            ins=[ins[w].opt()],
                            outs=[outs[w].opt()],
                        )

                    def atf_dma(w, j):
                        """Fetch gathered at_full half-window into SBUF.
                        c_in chunks 0:2 come from the h2 gather, 2:6 from
                        the h01 gather (w_proj rows are permuted to match).
                        """
                        js = slice(j * FW, (j + 1) * FW)
                        nc.sync.dma_start(
                            out=atf_sb[w][:, 0:2, js],
                            in_=ag2_outs[w][:, js].rearrange(
                                "(c p) n -> p c n", p=128))
                        nc.sync.dma_start(
                            out=atf_sb[w][:, 2:CT, js],
                            in_=ag01_outs[w][:, js].rearrange(
                                "(c p) n -> p c n", p=128))

                    def proj_m(w, m):
                        """out^T[m-chunk of local c-slice, window w]."""
                        mlo = m * 128
                        msz = min(128, CHL - mlo)
                        pr = PS.tile([msz, QW], F32, tag="sc", bufs=2,
                                     padded_shape=[128, QW], name="pr")
                        for j in range(QW // FW):
                            ps_js = slice(j * FW, (j + 1) * FW)
                            for kc in range(CT):
                                nc.tensor.matmul(
                                    pr[:, ps_js],
                                    lhsT=wp_sb[:, kc, mlo:mlo + msz],
                                    rhs=atf_sb[w][:, kc, ps_js],
                                    start=(kc == 0), stop=(kc == CT - 1),
                                )
                        po = AS.tile([msz, QW], F16, tag="po", bufs=3)
                        if w == 1 and m == 0:
                            # at the tail ACT is idle: it takes one m-chunk's
                            # bias-add while DVE takes the other, in parallel
                            # (mid-kernel ACT is exp-saturated, so w0 stays
                            # off the scalar queue)
                            nc.scalar.activation(po[:], pr[:], AF.Identity,
                                                 bias=bp_sb[0:msz, m:m + 1])
                        else:
                            nc.vector.tensor_scalar_add(
                                po[:], pr[:], bp_sb[0:msz, m:m + 1])
                        nc.sync.dma_start(
                            out=out_d[mlo:mlo + msz, w * QW:(w + 1) * QW],
                            in_=po[:])

                    def attn_h2(w, interleave):
                        """Head 2, adjacent-k-chunk row-tile-paired.
                        av_accums trail the scores/exp stream by one kcp so
                        the in-order PE queue never head-of-line blocks on
                        the accumulator psum slot (which waits for the
                        previous phase's normalize) at a phase seam."""
                        A2 = PS.tile([VW, QW], F32, tag="ac", bufs=2,
                                     padded_shape=[128, QW], name=f"A2_{w}")
                        pend = []
                        for kcp in range(KT // 2):
                            kc0, kc1 = 2 * kcp, 2 * kcp + 1
                            Se = PS.tile([128, QW], F32, tag="sc", bufs=2)
                            So = PS.tile([128, QW], F32, tag="sc", bufs=2)
                            scores_pair(w, k2d_sb, q2d_sb, kc0, kc1, Se, So)
                            Ee = AS.tile([128, QW], F16, tag="E", bufs=10)
                            Eo = AS.tile([128, QW], F16, tag="E", bufs=10)
                            nc.scalar.activation(Ee[:], Se[:], AF.Exp,
                                                 scale=SCALE)
                            nc.scalar.activation(Eo[:], So[:], AF.Exp,
                                                 scale=SCALE)
                            pend += [(Ee, kc0), (Eo, kc1)]
                            interleave(kcp)
                            while len(pend) > 2:
                                E, kc = pend.pop(0)
                                av_accum(A2, E, kc, 2, kc == 0, kc == KT - 1)
                        for E, kc in pend:
                            av_accum(A2, E, kc, 2, kc == 0, kc == KT - 1)
                        return A2

                    def attn_h01(w, interleave):
                        """Heads 0/1, head-row-tile-paired; accums trail by
                        two k-chunks (see attn_h2)."""
                        A0 = PS.tile([VW, QW], F32, tag="ac", bufs=2,
                                     padded_shape=[128, QW], name=f"A0_{w}")
                        A1 = PS.tile([VW, QW], F32, tag="ac", bufs=2,
                                     padded_shape=[128, QW], name=f"A1_{w}")
                        pend = []
                        for kc in range(KT):
                            S0 = PS.tile([128, QW], F32, tag="sc", bufs=2)
                            S1 = PS.tile([128, QW], F32, tag="sc", bufs=2)
                            scores_pair(w, k01_sb, q01_sb, kc, kc, S0, S1)
                            E0 = AS.tile([128, QW], F16, tag="E", bufs=10)
                            E1 = AS.tile([128, QW], F16, tag="E", bufs=10)
                            nc.scalar.activation(E0[:], S0[:], AF.Exp,
                                                 scale=SCALE)
                            nc.scalar.activation(E1[:], S1[:], AF.Exp,
                                                 scale=SCALE)
                            pend.append((E0, E1, kc))
                            interleave(kc)
                            while len(pend) > 2:
                                E0p, E1p, kcp_ = pend.pop(0)
                                av_accum(A0, E0p, kcp_, 0, kcp_ == 0,
                                         kcp_ == KT - 1)
                                av_accum(A1, E1p, kcp_, 1, kcp_ == 0,
                                         kcp_ == KT - 1)
                        for E0p, E1p, kcp_ in pend:
                            av_accum(A0, E0p, kcp_, 0, kcp_ == 0,
                                     kcp_ == KT - 1)
                            av_accum(A1, E1p, kcp_, 1, kcp_ == 0,
                                     kcp_ == KT - 1)
                        return A0, A1

                    # phase A: w0 head 2; all v chunks + phase-B q/k deps as
                    # fillers.  Accums for kc pair j flush at iteration j+1,
                    # after interleave(j+1), so v[2j]/v[2j+1] (supplied at
                    # iteration j) always precede their consumption.
                    def fillA(kcp):
                        v_group(2 * kcp)
                        v_group(2 * kcp + 1)
                        if kcp < 3:
                            # k2d f(kcp+1): consumed by score pairs at
                            # iteration 2*(kcp+1), two iterations later
                            qk_group(k2d_sb, wka_sb, 128, kcp + 1)
                        elif kcp == 3:
                            qk_group(k01_sb, wka_sb, 0, 0)
                        elif kcp == 4:
                            qk_group(q01_sb, wqa_sb, 0, 0)
                        elif kcp == 5:
                            qk_group(q01_sb, wqa_sb, 0, 1)
                    A2 = attn_h2(0, fillA)
                    # phase B: w0 heads 0/1; rest of the q/k projections.
                    # AG(w0-h2) fires at phase B start and hides here.
                    normalize(A2, 0, 2)
                    all_gather(0, 2)
                    fillB = [
                        lambda: qk_group(k01_sb, wka_sb, 0, 1),
                        lambda: qk_group(k01_sb, wka_sb, 0, 2),
                        lambda: qk_group(k01_sb, wka_sb, 0, 3),
                        lambda: qk_group(q01_sb, wqa_sb, 0, 2),
                        lambda: qk_group(q01_sb, wqa_sb, 0, 3),
                        lambda: qk_group(q2d_sb, wqa_sb, 128, 2),
                        lambda: qk_group(q2d_sb, wqa_sb, 128, 3),
                    ]
                    A0, A1 = attn_h01(
                        0, lambda kc: fillB[kc]() if kc < len(fillB) else None)
                    # phase C: w1 head 2; AG(w0-h01) fires here and hides
                    # under phase C
                    normalize(A0, 0, 0)
                    normalize(A1, 0, 1)
                    all_gather(0, 0)
                    A2 = attn_h2(1, lambda kcp: None)
                    # phase D: w1 heads 0/1; AG(w1-h2) hides here and the
                    # w0 projection runs as fillers once AG(w0) has landed
                    normalize(A2, 1, 2)
                    all_gather(1, 2)

                    def fillD(kc):
                        if kc == 2:
                            for j in range(QW // FW):
                                atf_dma(0, j)
                        elif kc == 5:
                            proj_m(0, 0)
                        elif kc == 9:
                            proj_m(0, 1)
                    A0, A1 = attn_h01(1, fillD)

                    # ---- tail: normalize w1 h01, AllGather, local proj ----
                    # Latency-optimized normalize: the two heads' chains are
                    # issued interleaved, the denominator reciprocal row is
                    # partition-broadcast on the (idle) PE instead of
                    # gpsimd, and the final multiplies split across DVE and
                    # gpsimd so they run in parallel.
                    dens, bcss, rcps = [], [], []
                    for h, A in ((0, A0), (1, A1)):
                        den = AS.tile([1, QW], F16, tag="dent", bufs=2,
                                      name=f"dent{h}")
                        nc.vector.tensor_copy(den[:], A[64:65, :])
                        dens.append(den)
                    for h, A in ((0, A0), (1, A1)):
                        bcs = PS.tile([64, QW], F32, tag="sc", bufs=2,
                                      padded_shape=[128, QW], name=f"bcst{h}")
                        for j in range(QW // FW):
                            js = slice(j * FW, (j + 1) * FW)
                            nc.tensor.matmul(bcs[:, js], lhsT=ones64[:],
                                             rhs=dens[h][:, js])
                        bcss.append(bcs)
                    for h, A in ((0, A0), (1, A1)):
                        rcp = AS.tile([64, QW], F32, tag="rcpt", bufs=2,
                                      name=f"rcpt{h}")
                        for j in range(QW // FW):
                            js = slice(j * FW, (j + 1) * FW)
                            nc.vector.reciprocal_approx_fast(
                                rcp[:, js], bcss[h][:, js])
                        rcps.append(rcp)
                    for h, A in ((0, A0), (1, A1)):
                        at = AS.tile([64, QW], F16, tag="at", bufs=3,
                                     name=f"att{h}")
                        for j in range(QW // FW):
                            js = slice(j * FW, (j + 1) * FW)
                            nc.vector.tensor_mul(at[:, js], A[0:64, js],
                                                 rcps[h][:, js])
                        nc.sync.dma_start(
                            out=ag01_ins[1][h * 64:(h + 1) * 64, :],
                            in_=at[:])
                    all_gather(1, 0)
                    # dummy burst bridges the PE across the AG wait so the
                    # HAM un-throttle is re-armed for the projection.
                    dumT = PS.tile([128, QW], F32, tag="sc", bufs=2)
                    dummy_burst(dumT, 6)
                    for j in range(QW // FW):
                        atf_dma(1, j)
                    proj_m(1, 0)
                    proj_m(1, 1)
    nc.finalize()
    return nc


def get_nc():
    if "nc" not in _CACHE:
        _CACHE["nc"] = _build_nc()
    return _CACHE["nc"]


def make_in_maps(x, w_qkv, w_proj, b_proj):
    x = np.asarray(x, dtype=np.float32)
    w_qkv = np.asarray(w_qkv, dtype=np.float32)
    w_proj = np.asarray(w_proj, dtype=np.float32)
    b_proj = np.asarray(b_proj, dtype=np.float32)
    in_maps = []
    for core in range(NCORES):
        b, g = divmod(core, G)
        cs = slice(g * CHL, (g + 1) * CHL)
        wq = w_qkv[:, 0 * C:1 * C][:, cs]
        wk = w_qkv[:, 1 * C:2 * C][:, cs]
        wv = w_qkv[:, 2 * C:3 * C][:, cs]
        # [heads01 | head2 | head2-dup]
        wqa = np.concatenate([wq[:, 0:128], wq[:, 128:192], wq[:, 128:192]],
                             axis=1)
        wka = np.concatenate([wk[:, 0:128], wk[:, 128:192], wk[:, 128:192]],
                             axis=1)
        # bias for the local c_out slice, [128, 2] column-per-m-chunk
        bp = np.zeros((128, 2), dtype=np.float32)
        bp[:, 0] = b_proj[cs][0:128]
        bp[0:64, 1] = b_proj[cs][128:192]
        # w_proj rows permuted to the gathered at_full order:
        # [4 groups' h2 | 4 groups' (h0, h1)]
        head_order = [2, 5, 8, 11, 0, 1, 3, 4, 6, 7, 9, 10]
        row_perm = np.concatenate(
            [np.arange(h * HD, (h + 1) * HD) for h in head_order])
        im = {
            "xT": np.ascontiguousarray(x[b].T, dtype=np.float16),
            "wqa": np.ascontiguousarray(wqa, dtype=np.float16),
            "wka": np.ascontiguousarray(wka, dtype=np.float16),
            "wv": np.ascontiguousarray(wv, dtype=np.float16),
            "wp": np.ascontiguousarray(w_proj[row_perm][:, cs],
                                       dtype=np.float16),
            "bp": bp,
        }
        in_maps.append(im)
    return in_maps


def unshard(results):
    out = np.empty((B, N, C), dtype=np.float32)
    for b in range(B):
        outT = np.concatenate(
            [np.asarray(results[b * G + g]["out"], dtype=np.float32)
             for g in range(G)], axis=0)
        out[b] = outT.T
    return out


def kernel(x, w_qkv, w_proj, b_proj):
    from concourse.bass_utils import run_bass_kernel_spmd

    nc = get_nc()
    in_maps = make_in_maps(x, w_qkv, w_proj, b_proj)
    res = run_bass_kernel_spmd(nc, in_maps, list(range(NCORES)))
    return unshard(res.results)
